# revision 1
# baseline (speedup 1.0000x reference)
"""Grouped MLP (MoE expert MLP, ragged token groups) on 8 TRN2 NeuronCores.

Strategy: token-parallel. The T tokens are grouped contiguously by expert;
we pad each expert's group to a multiple of CHUNK tokens, giving a list of
fixed-size chunks, each owned by exactly one expert. Chunks are assigned
contiguously and near-evenly to the 8 cores. Each core runs the same SPMD
program: for each of S weight "slots" (expert segments), load that expert's
w1/w2 (bf16) into SBUF, then a dynamic-trip-count loop over that segment's
chunks computes  yT = w2T_stat @ gelu(w1T_stat @ xT)  entirely in the
transposed [feature, token] layout, so both GEMMs use the weights as the
PE's stationary operand and no on-device transposes are needed.

Host side: transpose/cast x to bf16 [H, tokens], gather per-slot weights,
scatter the fp32 output back. Compiled program is cached per (schedule
shape) so repeated calls only pay execution.
"""

import numpy as np
import ml_dtypes

import concourse.bass as bass
import concourse.mybir as mybir
import concourse.tile as tile
from concourse import bacc
from concourse.bass_utils import run_bass_kernel_spmd

# Problem shape (fixed by the task).
T, H, F, E = 16384, 1024, 4096, 8
NCORES = 8
CHUNK = 256           # tokens per chunk = matmul moving-dim N
HT = H // 128         # 8 h-tiles
FT = F // 128         # 32 f-tiles

_BF16 = mybir.dt.bfloat16
_F32 = mybir.dt.float32
_I32 = mybir.dt.int32

_cache = {}

# Overridable for CoreSim validation (sim doesn't implement Gelu).
GELU_FUNC = mybir.ActivationFunctionType.Gelu
# Inner chunk-loop back-edge style (perf knob).
STAGGERED = True


def _schedule(counts):
    """counts[E] -> (chunks, per-core chunk lists, per-core segments).

    chunk = (expert, global_token_start, valid_len<=CHUNK)
    segment = (expert, local_base_chunk, n_chunks)
    """
    starts = np.concatenate([[0], np.cumsum(counts)])
    chunks = []
    for e in range(E):
        c = int(counts[e])
        for off in range(0, max(c, 1), CHUNK):
            if c == 0:
                break
            chunks.append((e, int(starts[e]) + off, min(CHUNK, c - off)))
    n = len(chunks)
    q, r = divmod(n, NCORES)
    sizes = [q + 1] * r + [q] * (NCORES - r)
    percore = []
    pos = 0
    for s in sizes:
        percore.append(chunks[pos:pos + s])
        pos += s
    segs = []
    for lst in percore:
        s = []
        for j, (e, _, _) in enumerate(lst):
            if s and s[-1][0] == e:
                s[-1][2] += 1
            else:
                s.append([e, j, 1])
        segs.append(s)
    return chunks, percore, segs


def _build(n_slots, maxc):
    """Build + compile the SPMD program for n_slots weight slots and
    a per-core DRAM capacity of maxc chunks."""
    key = (n_slots, maxc, GELU_FUNC, STAGGERED)
    if key in _cache:
        return _cache[key]

    nc = bacc.Bacc("TRN2", target_bir_lowering=False, debug=False,
                   num_devices=NCORES)
    xt_d = nc.declare_dram_parameter("xt", [H, maxc * CHUNK], _BF16,
                                     isOutput=False)
    w1_d = nc.declare_dram_parameter("w1s", [n_slots, H, F], _BF16,
                                     isOutput=False)
    w2_d = nc.declare_dram_parameter("w2s", [n_slots, F, H], _BF16,
                                     isOutput=False)
    # meta[0, :2*n_slots] = per-slot (n_chunks, base); meta[0, -1] = reps
    # (outer repetition count — 1 for normal runs, >1 for on-device timing;
    # the program is idempotent so extra reps just recompute the output).
    meta_d = nc.declare_dram_parameter("meta", [1, 2 * n_slots + 1], _I32,
                                       isOutput=False)
    yt_d = nc.declare_dram_parameter("yt", [H, maxc * CHUNK], _F32,
                                     isOutput=True)

    xt_r = xt_d.rearrange("(ht p) m -> p ht m", p=128)
    yt_r = yt_d.rearrange("(ht p) m -> p ht m", p=128)

    with tile.TileContext(nc) as tc:
        with (
            tc.tile_pool(name="meta", bufs=1) as mpool,
            tc.tile_pool(name="w1", bufs=1) as w1pool,
            tc.tile_pool(name="w2", bufs=1) as w2pool,
            tc.tile_pool(name="x", bufs=3) as xpool,
            tc.tile_pool(name="act", bufs=2) as apool,
            tc.tile_pool(name="y", bufs=2) as ypool,
            tc.tile_pool(name="ps1", bufs=4, space="PSUM") as ps1pool,
            tc.tile_pool(name="ps2", bufs=4, space="PSUM") as ps2pool,
        ):
            mt = mpool.tile([1, 2 * n_slots + 1], _I32)
            nc.sync.dma_start(mt[:], meta_d[:])
            reps = nc.values_load(mt[:1, 2 * n_slots:2 * n_slots + 1],
                                  min_val=1, max_val=1000,
                                  skip_runtime_bounds_check=True)
            bounds = []
            for s in range(n_slots):
                # skip_runtime_bounds_check: runtime assert traps kill the
                # axon/PJRT execution path (no debugger to service them).
                n_s = nc.values_load(mt[:1, 2 * s:2 * s + 1],
                                     min_val=0, max_val=maxc,
                                     skip_runtime_bounds_check=True)
                b_s = nc.values_load(mt[:1, 2 * s + 1:2 * s + 2],
                                     min_val=0, max_val=max(maxc - 1, 0),
                                     skip_runtime_bounds_check=True)
                bounds.append((n_s, b_s))

            rep_loop = tc.For_i(0, reps, name="reps")
            rep_loop.__enter__()
            for s in range(n_slots):
                w1sb = w1pool.tile([128, HT, F], _BF16, tag="w1sb")
                nc.sync.dma_start(
                    w1sb[:], w1_d[s].rearrange("(ht p) f -> p ht f", p=128))
                w2sb = w2pool.tile([128, FT, H], _BF16, tag="w2sb")
                nc.sync.dma_start(
                    w2sb[:], w2_d[s].rearrange("(ft p) h -> p ft h", p=128))

                n_s, b_s = bounds[s]
                with tc.For_i(0, n_s, hint_engines=(mybir.EngineType.PE,),
                              staggered_reset=STAGGERED) as i:
                    moff = nc.s_assert_within(
                        (b_s + i) * CHUNK, min_val=0,
                        max_val=(maxc - 1) * CHUNK,
                        skip_runtime_assert=True)
                    xt_sb = xpool.tile([128, HT, CHUNK], _BF16, tag="xt")
                    nc.sync.dma_start(xt_sb[:],
                                      xt_r[:, :, bass.ds(moff, CHUNK)])
                    act_sb = apool.tile([128, FT, CHUNK], _BF16, tag="act")
                    for f in range(FT):
                        ps = ps1pool.tile([128, CHUNK], _F32, tag="ps1")
                        for h in range(HT):
                            nc.tensor.matmul(
                                ps[:],
                                w1sb[:, h, f * 128:(f + 1) * 128],
                                xt_sb[:, h],
                                start=(h == 0), stop=(h == HT - 1))
                        nc.scalar.activation(act_sb[:, f], ps[:], GELU_FUNC)
                    yt_sb = ypool.tile([128, HT, CHUNK], _F32, tag="yt")
                    for h in range(HT):
                        ps2 = ps2pool.tile([128, CHUNK], _F32, tag="ps2")
                        for f in range(FT):
                            nc.tensor.matmul(
                                ps2[:],
                                w2sb[:, f, h * 128:(h + 1) * 128],
                                act_sb[:, f],
                                start=(f == 0), stop=(f == FT - 1))
                        nc.vector.tensor_copy(yt_sb[:, h], ps2[:])
                    nc.sync.dma_start(yt_r[:, :, bass.ds(moff, CHUNK)],
                                      yt_sb[:])
            rep_loop.__exit__(None, None, None)
    nc.compile()
    _cache[key] = nc
    return nc


def _make_inputs(x, w1, w2, percore, segs, n_slots, maxc, reps=1):
    w1b = w1.astype(ml_dtypes.bfloat16)
    w2b = w2.astype(ml_dtypes.bfloat16)
    in_maps = []
    for c in range(NCORES):
        lst, sg = percore[c], segs[c]
        xt = np.zeros((H, maxc * CHUNK), ml_dtypes.bfloat16)
        for j, (e, g, v) in enumerate(lst):
            xt[:, j * CHUNK:j * CHUNK + v] = x[g:g + v].T
        w1s = np.zeros((n_slots, H, F), ml_dtypes.bfloat16)
        w2s = np.zeros((n_slots, F, H), ml_dtypes.bfloat16)
        meta = np.zeros((1, 2 * n_slots + 1), np.int32)
        meta[0, 2 * n_slots] = reps
        for s in range(n_slots):
            if s < len(sg):
                e, b, n = sg[s]
            else:
                e, b, n = sg[-1][0], 0, 0
            w1s[s] = w1b[e]
            w2s[s] = w2b[e]
            meta[0, 2 * s] = n
            meta[0, 2 * s + 1] = b
        in_maps.append({"xt": xt, "w1s": w1s, "w2s": w2s, "meta": meta})
    return in_maps


def _gather(results, percore):
    out = np.zeros((T, H), np.float32)
    for c in range(NCORES):
        yt = results[c]["yt"]
        for j, (e, g, v) in enumerate(percore[c]):
            out[g:g + v] = yt[:, j * CHUNK:j * CHUNK + v].T
    return out


def kernel(permuted_local_hidden_states, weight1, weight2, tokens_per_expert):
    x = np.asarray(permuted_local_hidden_states, np.float32)
    w1 = np.asarray(weight1, np.float32)
    w2 = np.asarray(weight2, np.float32)
    counts = np.asarray(tokens_per_expert).astype(np.int64)

    chunks, percore, segs = _schedule(counts)
    n_slots = max(max(len(s) for s in segs), 1)
    maxc = max(max(len(p) for p in percore), 1)

    nc = _build(n_slots, maxc)
    in_maps = _make_inputs(x, w1, w2, percore, segs, n_slots, maxc)
    res = run_bass_kernel_spmd(nc, in_maps, list(range(NCORES)))
    return _gather(res.results, percore)



# revision 2
# speedup vs baseline: 1.2899x; 1.2899x over previous
"""Grouped MLP (MoE expert MLP, ragged token groups) on 8 TRN2 NeuronCores.

Strategy: tensor-parallel over the intermediate dim F. Every core holds a
1/8 column-slice of every expert's w1 (and the matching row-slice of w2)
resident in SBUF for the whole kernel — 128 KiB/partition for all 8
experts — and processes ALL T tokens, computing a partial fc2 output
that the host sums across the 8 cores. This gives:
  * zero weight reloads / zero expert-segment stalls on device,
  * perfectly balanced cores (identical token stream on every core),
  * exact-size token chunks (no padding waste: chunk = min(512, rest)).

Per chunk of m tokens for expert e (all in the transposed [feat, tok]
layout so weights are the PE-stationary operand):
  ps1[f,1:m]  = sum_h w1sb[e][h-part, f-cols].T @ xt[h-part, m]   (4 f-tiles)
  act         = gelu(ps1)                  (Activation engine, bf16 out)
  ps2[h, :m]  = sum_f w2sb[e][f-part, h-cols].T @ act[f-part, m]  (8 h-tiles)
  yt          = bf16(ps2)                  (DVE copy)
Partial y leaves as bf16; the host upconverts (exact: bf16 is truncated
f32) and accumulates in f32.

Host side: pack x.T/weights as bf16, scatter nothing (chunks are exact so
xt is literally x.T), sum the 8 partial outputs. Compiled program cached
per chunk schedule.
"""

import numpy as np
import ml_dtypes

import concourse.bass as bass
import concourse.mybir as mybir
import concourse.tile as tile
from concourse import bacc
from concourse.bass_utils import run_bass_kernel_spmd

# Problem shape (fixed by the task).
T, H, F, E = 16384, 1024, 4096, 8
NCORES = 8
FS = F // NCORES      # per-core F slice = 512
HT = H // 128         # 8 h-tiles
FT = FS // 128        # 4 f-tiles per core
CHUNK = 512           # max tokens per chunk = matmul moving-dim N

_BF16 = mybir.dt.bfloat16
_F32 = mybir.dt.float32

_cache = {}

GELU_FUNC = mybir.ActivationFunctionType.Gelu


def _schedule(counts):
    """counts[E] -> list of (expert, token_start, m) with exact sizes."""
    chunks = []
    off = 0
    for e in range(E):
        c = int(counts[e])
        o = 0
        while o < c:
            m = min(CHUNK, c - o)
            chunks.append((e, off + o, m))
            o += m
        off += c
    assert off == sum(int(c) for c in counts)
    return chunks


def _build(chunks, total):
    """Build + compile the SPMD program for a static chunk schedule."""
    key = (tuple(chunks), total, GELU_FUNC)
    if key in _cache:
        return _cache[key]

    nc = bacc.Bacc("TRN2", target_bir_lowering=False, debug=False,
                   num_devices=NCORES)
    xt_d = nc.declare_dram_parameter("xt", [128, HT, total], _BF16,
                                     isOutput=False)
    w1_d = nc.declare_dram_parameter("w1s", [E, 128, HT, FS], _BF16,
                                     isOutput=False)
    w2_d = nc.declare_dram_parameter("w2s", [E, 128, FT, H], _BF16,
                                     isOutput=False)
    yt_d = nc.declare_dram_parameter("yt", [128, HT, total], _BF16,
                                     isOutput=True)

    experts_used = []
    for e, _, _ in chunks:
        if e not in experts_used:
            experts_used.append(e)

    with tile.TileContext(nc) as tc:
        with (
            tc.tile_pool(name="w1", bufs=1) as w1pool,
            tc.tile_pool(name="w2", bufs=1) as w2pool,
            tc.tile_pool(name="x", bufs=3) as xpool,
            tc.tile_pool(name="act", bufs=2) as apool,
            tc.tile_pool(name="y", bufs=2) as ypool,
            tc.tile_pool(name="ps1", bufs=3, space="PSUM") as ps1pool,
            tc.tile_pool(name="ps2", bufs=3, space="PSUM") as ps2pool,
        ):
            # All experts' weight slices stay SBUF-resident. Loaded in
            # first-use order on the gpsimd (Pool) SWDGE queue so they
            # never contend with the x/y streams.
            w1sb = {}
            w2sb = {}
            for e in experts_used:
                t1 = w1pool.tile([128, HT, FS], _BF16, tag=f"w1_{e}",
                                 name=f"w1sb{e}")
                nc.gpsimd.dma_start(t1[:], w1_d[e])
                t2 = w2pool.tile([128, FT, H], _BF16, tag=f"w2_{e}",
                                 name=f"w2sb{e}")
                nc.gpsimd.dma_start(t2[:], w2_d[e])
                w1sb[e] = t1
                w2sb[e] = t2

            for e, off, m in chunks:
                xt = xpool.tile([128, HT, CHUNK], _BF16, tag="xt")
                nc.sync.dma_start(xt[:, :, :m], xt_d[:, :, off:off + m])
                act = apool.tile([128, FT, CHUNK], _BF16, tag="act")
                for f in range(FT):
                    ps = ps1pool.tile([128, CHUNK], _F32, tag="ps1")
                    for h in range(HT):
                        nc.tensor.matmul(
                            ps[:, :m],
                            w1sb[e][:, h, f * 128:(f + 1) * 128],
                            xt[:, h, :m],
                            start=(h == 0), stop=(h == HT - 1))
                    nc.scalar.activation(act[:, f, :m], ps[:, :m], GELU_FUNC)
                yt = ypool.tile([128, HT, CHUNK], _BF16, tag="yt")
                for h in range(HT):
                    ps2 = ps2pool.tile([128, CHUNK], _F32, tag="ps2")
                    for f in range(FT):
                        nc.tensor.matmul(
                            ps2[:, :m],
                            w2sb[e][:, f, h * 128:(h + 1) * 128],
                            act[:, f, :m],
                            start=(f == 0), stop=(f == FT - 1))
                    nc.vector.tensor_copy(yt[:, h, :m], ps2[:, :m])
                nc.scalar.dma_start(yt_d[:, :, off:off + m], yt[:, :, :m])
    nc.compile()
    _cache[key] = nc
    return nc


def _make_inputs(x, w1, w2):
    """Per-core input maps. xt is shared (x.T packed); weights per core."""
    xt = np.ascontiguousarray(
        x.astype(ml_dtypes.bfloat16).T.reshape(HT, 128, T).transpose(1, 0, 2))
    w1b = w1.astype(ml_dtypes.bfloat16)
    w2b = w2.astype(ml_dtypes.bfloat16)
    in_maps = []
    for c in range(NCORES):
        w1s = np.ascontiguousarray(
            w1b[:, :, c * FS:(c + 1) * FS]
            .reshape(E, HT, 128, FS).transpose(0, 2, 1, 3))
        w2s = np.ascontiguousarray(
            w2b[:, c * FS:(c + 1) * FS, :]
            .reshape(E, FT, 128, H).transpose(0, 2, 1, 3))
        in_maps.append({"xt": xt, "w1s": w1s, "w2s": w2s})
    return in_maps


def _gather(results):
    """Sum 8 bf16 partial outputs in f32 and restore [T, H] layout."""
    acc = np.zeros((128, HT, T), np.float32)
    for c in range(NCORES):
        yb = results[c]["yt"]
        # bf16 -> f32 exactly via bit shift (bf16 is truncated f32)
        acc += (yb.view(np.uint16).astype(np.uint32) << 16).view(np.float32)
    return np.ascontiguousarray(
        acc.transpose(1, 0, 2).reshape(H, T).T)


def kernel(permuted_local_hidden_states, weight1, weight2, tokens_per_expert):
    x = np.asarray(permuted_local_hidden_states, np.float32)
    w1 = np.asarray(weight1, np.float32)
    w2 = np.asarray(weight2, np.float32)
    counts = np.asarray(tokens_per_expert).astype(np.int64)

    chunks = _schedule(counts)
    nc = _build(chunks, int(counts.sum()))
    in_maps = _make_inputs(x, w1, w2)
    res = run_bass_kernel_spmd(nc, in_maps, list(range(NCORES)))
    return _gather(res.results)


# revision 3
# speedup vs baseline: 1.4881x; 1.1536x over previous
"""Grouped MLP (MoE expert MLP, ragged token groups) on 8 TRN2 NeuronCores.

Strategy: tensor-parallel over the intermediate dim F. Every core holds a
1/8 column-slice of every expert's w1 (and the matching row-slice of w2)
resident in SBUF for the whole kernel — 128 KiB/partition for all 8
experts — and processes ALL T tokens, computing a partial fc2 output
that the host sums across the 8 cores. This gives:
  * zero weight reloads / zero expert-segment stalls on device,
  * perfectly balanced cores (identical token stream on every core),
  * exact-size token chunks (no padding waste: chunk = min(512, rest)).

Per chunk of m tokens for expert e (all in the transposed [feat, tok]
layout so weights are the PE-stationary operand):
  ps1[f, :m]  = sum_h w1sb[e][h-part, f-cols].T @ xt[h-part, m]   (4 f-tiles)
  act         = gelu(ps1)                  (Activation engine, bf16 out)
  ps2[h, :m]  = sum_f w2sb[e][f-part, h-cols].T @ act[f-part, m]  (8 h-tiles)
  yt          = bf16(ps2)                  (DVE cast)
Partial y leaves as bf16; the host upconverts (exact: bf16 is truncated
f32) and accumulates in f32.

DMA layout: x and y live in DRAM chunk-major and flat per partition
([128, sum(HT*m)]) so every chunk transfer is one contiguous 8KB-per-
partition descriptor — ~5x the per-engine DMA rate of strided 1KB
pieces. Weight loads are interleaved across the two spare DGE queues
(gpsimd SWDGE / Activation HWDGE) in first-use order so early experts'
weights land before the PE needs them; the first expert's tiles are
split in half for a faster pipeline start. The schedule ends with the
globally smallest chunk to minimize the post-matmul drain tail.

Host side: pack x.T/weights as bf16, sum the 8 partial outputs.
Compiled program cached per chunk schedule.
"""

import numpy as np
import ml_dtypes

import concourse.bass as bass
import concourse.mybir as mybir
import concourse.tile as tile
from concourse import bacc
from concourse.bass_utils import run_bass_kernel_spmd

# Problem shape (fixed by the task).
T, H, F, E = 16384, 1024, 4096, 8
NCORES = 8
FS = F // NCORES      # per-core F slice = 512
HT = H // 128         # 8 h-tiles
FT = FS // 128        # 4 f-tiles per core
CHUNK = 512           # max tokens per chunk = matmul moving-dim N

_BF16 = mybir.dt.bfloat16
_F32 = mybir.dt.float32

_cache = {}

GELU_FUNC = mybir.ActivationFunctionType.Gelu


def _schedule(counts):
    """counts[E] -> list of (expert, token_start, m) with exact sizes.

    Expert processing order puts the expert owning the globally smallest
    chunk last (and that chunk last within it) so the kernel's drain tail
    is as short as possible.
    """
    starts = np.concatenate([[0], np.cumsum([int(c) for c in counts])])
    per_e = []
    for e in range(E):
        c = int(counts[e])
        ch = []
        o = 0
        while o < c:
            m = min(CHUNK, c - o)
            ch.append((e, int(starts[e]) + o, m))
            o += m
        if ch:
            per_e.append(ch)
    if not per_e:
        return []
    # expert whose last (smallest) chunk is globally smallest goes last
    tail_i = min(range(len(per_e)), key=lambda i: per_e[i][-1][2])
    order = [i for i in range(len(per_e)) if i != tail_i] + [tail_i]
    chunks = []
    for i in order:
        chunks.extend(per_e[i])
    return chunks


def _build(chunks):
    """Build + compile the SPMD program for a static chunk schedule."""
    key = (tuple(chunks), GELU_FUNC)
    if key in _cache:
        return _cache[key]

    xtot = sum(HT * m for _, _, m in chunks)
    nc = bacc.Bacc("TRN2", target_bir_lowering=False, debug=False,
                   num_devices=NCORES)
    xt_d = nc.declare_dram_parameter("xt", [128, xtot], _BF16,
                                     isOutput=False)
    w1_d = nc.declare_dram_parameter("w1s", [E, 128, HT, FS], _BF16,
                                     isOutput=False)
    w2_d = nc.declare_dram_parameter("w2s", [E, 128, FT, H], _BF16,
                                     isOutput=False)
    yt_d = nc.declare_dram_parameter("yt", [128, xtot], _BF16,
                                     isOutput=True)

    experts_used = []
    for e, _, _ in chunks:
        if e not in experts_used:
            experts_used.append(e)

    with tile.TileContext(nc) as tc:
        with (
            tc.tile_pool(name="w1", bufs=1) as w1pool,
            tc.tile_pool(name="w2", bufs=1) as w2pool,
            tc.tile_pool(name="x", bufs=3) as xpool,
            tc.tile_pool(name="act", bufs=2) as apool,
            tc.tile_pool(name="y", bufs=3) as ypool,
            tc.tile_pool(name="ps1", bufs=3, space="PSUM") as ps1pool,
            tc.tile_pool(name="ps2", bufs=3, space="PSUM") as ps2pool,
        ):
            # All experts' weight slices stay SBUF-resident. Loads are
            # interleaved across the two spare DGE queues (gpsimd SWDGE,
            # Activation HWDGE) in first-use order; the first expert's
            # tiles are split so the pipeline starts sooner.
            w1sb = {}
            w2sb = {}
            for k, e in enumerate(experts_used):
                eng = nc.gpsimd if k % 2 == 0 else nc.scalar
                t1 = w1pool.tile([128, HT, FS], _BF16, tag=f"w1_{e}",
                                 name=f"w1sb{e}")
                t2 = w2pool.tile([128, FT, H], _BF16, tag=f"w2_{e}",
                                 name=f"w2sb{e}")
                if k == 0:
                    half = FS // 2
                    eng.dma_start(t1[:, :, :half], w1_d[e][:, :, :half])
                    eng.dma_start(t1[:, :, half:], w1_d[e][:, :, half:])
                    nc.scalar.dma_start(t2[:, :FT // 2], w2_d[e][:, :FT // 2])
                    nc.scalar.dma_start(t2[:, FT // 2:], w2_d[e][:, FT // 2:])
                else:
                    eng.dma_start(t1[:], w1_d[e])
                    eng.dma_start(t2[:], w2_d[e])
                w1sb[e] = t1
                w2sb[e] = t2

            base = 0
            for e, off, m in chunks:
                xt = xpool.tile([128, HT * CHUNK], _BF16, tag="xt")
                nc.sync.dma_start(xt[:, :HT * m], xt_d[:, base:base + HT * m])
                act = apool.tile([128, FT, CHUNK], _BF16, tag="act")
                for f in range(FT):
                    ps = ps1pool.tile([128, CHUNK], _F32, tag="ps1")
                    for h in range(HT):
                        nc.tensor.matmul(
                            ps[:, :m],
                            w1sb[e][:, h, f * 128:(f + 1) * 128],
                            xt[:, h * m:(h + 1) * m],
                            start=(h == 0), stop=(h == HT - 1))
                    nc.scalar.activation(act[:, f, :m], ps[:, :m], GELU_FUNC)
                yt = ypool.tile([128, HT * CHUNK], _BF16, tag="yt")
                for h in range(HT):
                    ps2 = ps2pool.tile([128, CHUNK], _F32, tag="ps2")
                    for f in range(FT):
                        nc.tensor.matmul(
                            ps2[:, :m],
                            w2sb[e][:, f, h * 128:(h + 1) * 128],
                            act[:, f, :m],
                            start=(f == 0), stop=(f == FT - 1))
                    nc.vector.tensor_copy(yt[:, h * m:(h + 1) * m],
                                          ps2[:, :m])
                nc.scalar.dma_start(yt_d[:, base:base + HT * m],
                                    yt[:, :HT * m])
                base += HT * m
    nc.compile()
    _cache[key] = nc
    return nc


def _make_inputs(x, w1, w2, chunks):
    """Per-core input maps. xt is shared (x.T packed chunk-major, flat);
    weights are per-core F-slices."""
    xtT = np.ascontiguousarray(
        x.astype(ml_dtypes.bfloat16).T.reshape(HT, 128, T).transpose(1, 0, 2))
    xt = np.concatenate(
        [xtT[:, :, off:off + m].reshape(128, HT * m) for _, off, m in chunks],
        axis=1)
    w1b = w1.astype(ml_dtypes.bfloat16)
    w2b = w2.astype(ml_dtypes.bfloat16)
    in_maps = []
    for c in range(NCORES):
        w1s = np.ascontiguousarray(
            w1b[:, :, c * FS:(c + 1) * FS]
            .reshape(E, HT, 128, FS).transpose(0, 2, 1, 3))
        w2s = np.ascontiguousarray(
            w2b[:, c * FS:(c + 1) * FS, :]
            .reshape(E, FT, 128, H).transpose(0, 2, 1, 3))
        in_maps.append({"xt": xt, "w1s": w1s, "w2s": w2s})
    return in_maps


def _gather(results, chunks):
    """Sum 8 bf16 partial outputs in f32 and restore [T, H] layout."""
    acc = np.zeros((128, HT, T), np.float32)
    for c in range(NCORES):
        yb = results[c]["yt"]
        # bf16 -> f32 exactly via bit shift (bf16 is truncated f32)
        yf = (yb.view(np.uint16).astype(np.uint32) << 16).view(np.float32)
        base = 0
        for _, off, m in chunks:
            acc[:, :, off:off + m] += yf[:, base:base + HT * m].reshape(
                128, HT, m)
            base += HT * m
    return np.ascontiguousarray(acc.transpose(1, 0, 2).reshape(H, T).T)


def kernel(permuted_local_hidden_states, weight1, weight2, tokens_per_expert):
    x = np.asarray(permuted_local_hidden_states, np.float32)
    w1 = np.asarray(weight1, np.float32)
    w2 = np.asarray(weight2, np.float32)
    counts = np.asarray(tokens_per_expert).astype(np.int64)

    chunks = _schedule(counts)
    nc = _build(chunks)
    in_maps = _make_inputs(x, w1, w2, chunks)
    res = run_bass_kernel_spmd(nc, in_maps, list(range(NCORES)))
    return _gather(res.results, chunks)


# revision 5
# speedup vs baseline: 1.5978x; 1.0737x over previous
"""Grouped MLP (MoE expert MLP, ragged token groups) on 8 TRN2 NeuronCores.

Strategy: tensor-parallel over the intermediate dim F. Every core holds a
1/8 column-slice of every expert's w1 (and the matching row-slice of w2)
resident in SBUF for the whole kernel — 128 KiB/partition for all 8
experts — and processes ALL T tokens, computing a partial fc2 output
that the host sums across the 8 cores. This gives:
  * zero weight reloads / zero expert-segment stalls on device,
  * perfectly balanced cores (identical token stream on every core),
  * exact-size token chunks (no padding waste: chunk = min(512, rest)).

Per chunk of m tokens for expert e (all in the transposed [feat, tok]
layout so weights are the PE-stationary operand):
  ps1[f, :m]  = sum_h w1sb[e][h-part, f-cols].T @ xt[h-part, m]   (4 f-tiles)
  act         = gelu(ps1)                  (Activation engine, bf16 out)
  ps2[h, :m]  = sum_f w2sb[e][f-part, h-cols].T @ act[f-part, m]  (8 h-tiles)
  yt          = bf16(ps2)                  (DVE cast)
Partial y leaves as bf16; the host upconverts (exact: bf16 is truncated
f32) and accumulates in f32.

DMA layout: x and y live in DRAM chunk-major and flat per partition
([128, sum(HT*m)]) so every chunk transfer is one contiguous 8KB-per-
partition descriptor — ~5x the per-engine DMA rate of strided 1KB
pieces. Weight loads are interleaved across the two spare DGE queues
(gpsimd SWDGE / Activation HWDGE) in first-use order so early experts'
weights land before the PE needs them; the first expert's tiles are
split in half for a faster pipeline start. The schedule ends with the
globally smallest chunk to minimize the post-matmul drain tail.

Host side: pack x.T/weights as bf16, sum the 8 partial outputs.
Compiled program cached per chunk schedule.
"""

import numpy as np
import ml_dtypes

import concourse.bass as bass
import concourse.mybir as mybir
import concourse.tile as tile
from concourse import bacc
from concourse.bass_utils import run_bass_kernel_spmd

# Problem shape (fixed by the task).
T, H, F, E = 16384, 1024, 4096, 8
NCORES = 8
FS = F // NCORES      # per-core F slice = 512
HT = H // 128         # 8 h-tiles
FT = FS // 128        # 4 f-tiles per core
CHUNK = 512           # max tokens per chunk = matmul moving-dim N

_BF16 = mybir.dt.bfloat16
_F32 = mybir.dt.float32

_cache = {}

GELU_FUNC = mybir.ActivationFunctionType.Gelu


def _schedule(counts):
    """counts[E] -> list of (expert, token_start, m) with exact sizes.

    Expert processing order puts the expert owning the globally smallest
    chunk last (and that chunk last within it) so the kernel's drain tail
    is as short as possible.
    """
    starts = np.concatenate([[0], np.cumsum([int(c) for c in counts])])
    per_e = []
    for e in range(E):
        c = int(counts[e])
        ch = []
        o = 0
        while o < c:
            m = min(CHUNK, c - o)
            ch.append((e, int(starts[e]) + o, m))
            o += m
        if ch:
            per_e.append(ch)
    if not per_e:
        return []
    # Biggest expert first: its chunks cover the FIFO drain of the single
    # weight-DMA queue, so later experts' weights always arrive in time.
    per_e.sort(key=lambda ch: -len(ch))
    # Expert whose last (smallest) chunk is globally smallest goes last so
    # the drain tail after the final matmul is minimal.
    if len(per_e) > 1:
        tail_i = min(range(1, len(per_e)), key=lambda i: per_e[i][-1][2])
        per_e.append(per_e.pop(tail_i))
    chunks = []
    for ch in per_e:
        chunks.extend(ch)
    return chunks


def _build(chunks):
    """Build + compile the SPMD program for a static chunk schedule."""
    key = (tuple(chunks), GELU_FUNC)
    if key in _cache:
        return _cache[key]

    xtot = sum(HT * m for _, _, m in chunks)
    nc = bacc.Bacc("TRN2", target_bir_lowering=False, debug=False,
                   num_devices=NCORES)
    xt_d = nc.declare_dram_parameter("xt", [128, xtot], _BF16,
                                     isOutput=False)
    w1_d = nc.declare_dram_parameter("w1s", [E, 128, HT, FS], _BF16,
                                     isOutput=False)
    w2_d = nc.declare_dram_parameter("w2s", [E, 128, FT, H], _BF16,
                                     isOutput=False)
    yt_d = nc.declare_dram_parameter("yt", [128, xtot], _BF16,
                                     isOutput=True)

    experts_used = []
    for e, _, _ in chunks:
        if e not in experts_used:
            experts_used.append(e)

    with tile.TileContext(nc) as tc:
        with (
            tc.tile_pool(name="w1", bufs=1) as w1pool,
            tc.tile_pool(name="w2", bufs=1) as w2pool,
            tc.tile_pool(name="x", bufs=3) as xpool,
            tc.tile_pool(name="act", bufs=2) as apool,
            tc.tile_pool(name="y", bufs=3) as ypool,
            tc.tile_pool(name="ps1", bufs=3, space="PSUM") as ps1pool,
            tc.tile_pool(name="ps2", bufs=3, space="PSUM") as ps2pool,
        ):
            # All experts' weight slices stay SBUF-resident, loaded on the
            # otherwise-idle gpsimd SWDGE queue ONLY (DMA queues drain
            # FIFO; mixing streams in a queue delays early items). First-
            # use order + biggest-expert-first schedule means each
            # expert's weights land well before the PE reaches them.
            w1sb = {}
            w2sb = {}
            for e in experts_used:
                t1 = w1pool.tile([128, HT, FS], _BF16, tag=f"w1_{e}",
                                 name=f"w1sb{e}")
                nc.gpsimd.dma_start(t1[:], w1_d[e])
                t2 = w2pool.tile([128, FT, H], _BF16, tag=f"w2_{e}",
                                 name=f"w2sb{e}")
                nc.gpsimd.dma_start(t2[:], w2_d[e])
                w1sb[e] = t1
                w2sb[e] = t2

            base = 0
            for e, off, m in chunks:
                xt = xpool.tile([128, HT * CHUNK], _BF16, tag="xt")
                nc.sync.dma_start(xt[:, :HT * m], xt_d[:, base:base + HT * m])
                act = apool.tile([128, FT, CHUNK], _BF16, tag="act")
                for f in range(FT):
                    ps = ps1pool.tile([128, CHUNK], _F32, tag="ps1")
                    for h in range(HT):
                        nc.tensor.matmul(
                            ps[:, :m],
                            w1sb[e][:, h, f * 128:(f + 1) * 128],
                            xt[:, h * m:(h + 1) * m],
                            start=(h == 0), stop=(h == HT - 1))
                    nc.scalar.activation(act[:, f, :m], ps[:, :m], GELU_FUNC)
                yt = ypool.tile([128, HT * CHUNK], _BF16, tag="yt")
                for h in range(HT):
                    ps2 = ps2pool.tile([128, CHUNK], _F32, tag="ps2")
                    for f in range(FT):
                        nc.tensor.matmul(
                            ps2[:, :m],
                            w2sb[e][:, f, h * 128:(h + 1) * 128],
                            act[:, f, :m],
                            start=(f == 0), stop=(f == FT - 1))
                    nc.vector.tensor_copy(yt[:, h * m:(h + 1) * m],
                                          ps2[:, :m])
                nc.scalar.dma_start(yt_d[:, base:base + HT * m],
                                    yt[:, :HT * m])
                base += HT * m
    nc.compile()
    _cache[key] = nc
    return nc


def _make_inputs(x, w1, w2, chunks):
    """Per-core input maps. xt is shared (x.T packed chunk-major, flat);
    weights are per-core F-slices."""
    xtT = np.ascontiguousarray(
        x.astype(ml_dtypes.bfloat16).T.reshape(HT, 128, T).transpose(1, 0, 2))
    xt = np.concatenate(
        [xtT[:, :, off:off + m].reshape(128, HT * m) for _, off, m in chunks],
        axis=1)
    w1b = w1.astype(ml_dtypes.bfloat16)
    w2b = w2.astype(ml_dtypes.bfloat16)
    in_maps = []
    for c in range(NCORES):
        w1s = np.ascontiguousarray(
            w1b[:, :, c * FS:(c + 1) * FS]
            .reshape(E, HT, 128, FS).transpose(0, 2, 1, 3))
        w2s = np.ascontiguousarray(
            w2b[:, c * FS:(c + 1) * FS, :]
            .reshape(E, FT, 128, H).transpose(0, 2, 1, 3))
        in_maps.append({"xt": xt, "w1s": w1s, "w2s": w2s})
    return in_maps


def _gather(results, chunks):
    """Sum 8 bf16 partial outputs in f32 and restore [T, H] layout."""
    acc = np.zeros((128, HT, T), np.float32)
    for c in range(NCORES):
        yb = results[c]["yt"]
        # bf16 -> f32 exactly via bit shift (bf16 is truncated f32)
        yf = (yb.view(np.uint16).astype(np.uint32) << 16).view(np.float32)
        base = 0
        for _, off, m in chunks:
            acc[:, :, off:off + m] += yf[:, base:base + HT * m].reshape(
                128, HT, m)
            base += HT * m
    return np.ascontiguousarray(acc.transpose(1, 0, 2).reshape(H, T).T)


def kernel(permuted_local_hidden_states, weight1, weight2, tokens_per_expert):
    x = np.asarray(permuted_local_hidden_states, np.float32)
    w1 = np.asarray(weight1, np.float32)
    w2 = np.asarray(weight2, np.float32)
    counts = np.asarray(tokens_per_expert).astype(np.int64)

    chunks = _schedule(counts)
    nc = _build(chunks)
    in_maps = _make_inputs(x, w1, w2, chunks)
    res = run_bass_kernel_spmd(nc, in_maps, list(range(NCORES)))
    return _gather(res.results, chunks)


# revision 7
# speedup vs baseline: 1.5998x; 1.0013x over previous
"""Grouped MLP (MoE expert MLP, ragged token groups) on 8 TRN2 NeuronCores.

Strategy: tensor-parallel over the intermediate dim F. Every core holds a
1/8 column-slice of every expert's w1 (and the matching row-slice of w2)
resident in SBUF for the whole kernel — 128 KiB/partition for all 8
experts — and processes ALL T tokens, computing a partial fc2 output
that the host sums across the 8 cores. This gives:
  * zero weight reloads / zero expert-segment stalls on device,
  * perfectly balanced cores (identical token stream on every core),
  * exact-size token chunks (no padding waste: chunk = min(512, rest)).

Per chunk of m tokens for expert e (all in the transposed [feat, tok]
layout so weights are the PE-stationary operand):
  ps1[f, :m]  = sum_h w1sb[e][h-part, f-cols].T @ xt[h-part, m]   (4 f-tiles)
  act         = gelu(ps1)                  (Activation engine, bf16 out)
  ps2[h, :m]  = sum_f w2sb[e][f-part, h-cols].T @ act[f-part, m]  (8 h-tiles)
  yt          = bf16(ps2)                  (DVE cast)
Partial y leaves as bf16; the host upconverts (exact: bf16 is truncated
f32) and accumulates in f32.

DMA layout: x and y live in DRAM chunk-major and flat per partition
([128, sum(HT*m)]) so every chunk transfer is one contiguous 8KB-per-
partition descriptor — ~5x the per-engine DMA rate of strided 1KB
pieces. Weight loads are interleaved across the two spare DGE queues
(gpsimd SWDGE / Activation HWDGE) in first-use order so early experts'
weights land before the PE needs them; the first expert's tiles are
split in half for a faster pipeline start. The schedule ends with the
globally smallest chunk to minimize the post-matmul drain tail.

Host side: pack x.T/weights as bf16, sum the 8 partial outputs.
Compiled program cached per chunk schedule.
"""

import numpy as np
import ml_dtypes

import concourse.bass as bass
import concourse.mybir as mybir
import concourse.tile as tile
from concourse import bacc
from concourse.bass_utils import run_bass_kernel_spmd

# Problem shape (fixed by the task).
T, H, F, E = 16384, 1024, 4096, 8
NCORES = 8
FS = F // NCORES      # per-core F slice = 512
HT = H // 128         # 8 h-tiles
FT = FS // 128        # 4 f-tiles per core
CHUNK = 512           # max tokens per chunk = matmul moving-dim N

_BF16 = mybir.dt.bfloat16
_F32 = mybir.dt.float32

_cache = {}

GELU_FUNC = mybir.ActivationFunctionType.Gelu


def _schedule(counts):
    """counts[E] -> list of (expert, token_start, m) with exact sizes.

    Expert processing order puts the expert owning the globally smallest
    chunk last (and that chunk last within it) so the kernel's drain tail
    is as short as possible.
    """
    starts = np.concatenate([[0], np.cumsum([int(c) for c in counts])])
    per_e = []
    for e in range(E):
        c = int(counts[e])
        ch = []
        o = 0
        while o < c:
            m = min(CHUNK, c - o)
            ch.append((e, int(starts[e]) + o, m))
            o += m
        if ch:
            per_e.append(ch)
    if not per_e:
        return []
    # Biggest expert first: its chunks cover the FIFO drain of the single
    # weight-DMA queue, so later experts' weights always arrive in time.
    per_e.sort(key=lambda ch: -len(ch))
    # Expert whose last (smallest) chunk is globally smallest goes last so
    # the drain tail after the final matmul is minimal.
    if len(per_e) > 1:
        tail_i = min(range(1, len(per_e)), key=lambda i: per_e[i][-1][2])
        per_e.append(per_e.pop(tail_i))
    chunks = []
    for ch in per_e:
        chunks.extend(ch)
    return chunks


def _build(chunks):
    """Build + compile the SPMD program for a static chunk schedule."""
    key = (tuple(chunks), GELU_FUNC)
    if key in _cache:
        return _cache[key]

    xtot = sum(HT * m for _, _, m in chunks)
    nc = bacc.Bacc("TRN2", target_bir_lowering=False, debug=False,
                   num_devices=NCORES)
    xt_d = nc.declare_dram_parameter("xt", [128, xtot], _BF16,
                                     isOutput=False)
    w1_d = nc.declare_dram_parameter("w1s", [E, 128, HT, FS], _BF16,
                                     isOutput=False)
    w2_d = nc.declare_dram_parameter("w2s", [E, 128, FT, H], _BF16,
                                     isOutput=False)
    yt_d = nc.declare_dram_parameter("yt", [128, xtot], _BF16,
                                     isOutput=True)

    experts_used = []
    for e, _, _ in chunks:
        if e not in experts_used:
            experts_used.append(e)

    with tile.TileContext(nc) as tc:
        with (
            tc.tile_pool(name="w1", bufs=1) as w1pool,
            tc.tile_pool(name="w2", bufs=1) as w2pool,
            tc.tile_pool(name="x", bufs=3) as xpool,
            tc.tile_pool(name="act", bufs=2) as apool,
            tc.tile_pool(name="y", bufs=3) as ypool,
            tc.tile_pool(name="ps1", bufs=3, space="PSUM") as ps1pool,
            tc.tile_pool(name="ps2", bufs=3, space="PSUM") as ps2pool,
        ):
            # All experts' weight slices stay SBUF-resident, loaded on the
            # otherwise-idle gpsimd SWDGE queue ONLY (DMA queues drain
            # FIFO; mixing streams in a queue delays early items). First-
            # use order + biggest-expert-first schedule means each
            # expert's weights land well before the PE reaches them.
            w1sb = {}
            w2sb = {}
            for k, e in enumerate(experts_used):
                t1 = w1pool.tile([128, HT, FS], _BF16, tag=f"w1_{e}",
                                 name=f"w1sb{e}")
                if k == 0:
                    # split so fc1 f-tile 0 can start before the rest lands
                    half = FS // 2
                    nc.gpsimd.dma_start(t1[:, :, :half], w1_d[e][:, :, :half])
                    nc.gpsimd.dma_start(t1[:, :, half:], w1_d[e][:, :, half:])
                else:
                    nc.gpsimd.dma_start(t1[:], w1_d[e])
                t2 = w2pool.tile([128, FT, H], _BF16, tag=f"w2_{e}",
                                 name=f"w2sb{e}")
                nc.gpsimd.dma_start(t2[:], w2_d[e])
                w1sb[e] = t1
                w2sb[e] = t2

            base = 0
            first_chunk = True
            for e, off, m in chunks:
                xt = xpool.tile([128, HT * CHUNK], _BF16, tag="xt")
                if first_chunk:
                    # split so the first fc1 accumulation (h 0..3) can
                    # start while the second half is still in flight
                    hh = (HT // 2) * m
                    nc.sync.dma_start(xt[:, :hh], xt_d[:, base:base + hh])
                    nc.sync.dma_start(xt[:, hh:HT * m],
                                      xt_d[:, base + hh:base + HT * m])
                    first_chunk = False
                else:
                    nc.sync.dma_start(xt[:, :HT * m],
                                      xt_d[:, base:base + HT * m])
                act = apool.tile([128, FT, CHUNK], _BF16, tag="act")
                for f in range(FT):
                    ps = ps1pool.tile([128, CHUNK], _F32, tag="ps1")
                    for h in range(HT):
                        nc.tensor.matmul(
                            ps[:, :m],
                            w1sb[e][:, h, f * 128:(f + 1) * 128],
                            xt[:, h * m:(h + 1) * m],
                            start=(h == 0), stop=(h == HT - 1))
                    nc.scalar.activation(act[:, f, :m], ps[:, :m], GELU_FUNC)
                yt = ypool.tile([128, HT * CHUNK], _BF16, tag="yt")
                for h in range(HT):
                    ps2 = ps2pool.tile([128, CHUNK], _F32, tag="ps2")
                    for f in range(FT):
                        nc.tensor.matmul(
                            ps2[:, :m],
                            w2sb[e][:, f, h * 128:(h + 1) * 128],
                            act[:, f, :m],
                            start=(f == 0), stop=(f == FT - 1))
                    nc.vector.tensor_copy(yt[:, h * m:(h + 1) * m],
                                          ps2[:, :m])
                nc.scalar.dma_start(yt_d[:, base:base + HT * m],
                                    yt[:, :HT * m])
                base += HT * m
    nc.compile()
    _cache[key] = nc
    return nc


def _make_inputs(x, w1, w2, chunks):
    """Per-core input maps. xt is shared (x.T packed chunk-major, flat);
    weights are per-core F-slices."""
    xtT = np.ascontiguousarray(
        x.astype(ml_dtypes.bfloat16).T.reshape(HT, 128, T).transpose(1, 0, 2))
    xt = np.concatenate(
        [xtT[:, :, off:off + m].reshape(128, HT * m) for _, off, m in chunks],
        axis=1)
    w1b = w1.astype(ml_dtypes.bfloat16)
    w2b = w2.astype(ml_dtypes.bfloat16)
    in_maps = []
    for c in range(NCORES):
        w1s = np.ascontiguousarray(
            w1b[:, :, c * FS:(c + 1) * FS]
            .reshape(E, HT, 128, FS).transpose(0, 2, 1, 3))
        w2s = np.ascontiguousarray(
            w2b[:, c * FS:(c + 1) * FS, :]
            .reshape(E, FT, 128, H).transpose(0, 2, 1, 3))
        in_maps.append({"xt": xt, "w1s": w1s, "w2s": w2s})
    return in_maps


def _gather(results, chunks):
    """Sum 8 bf16 partial outputs in f32 and restore [T, H] layout."""
    acc = np.zeros((128, HT, T), np.float32)
    for c in range(NCORES):
        yb = results[c]["yt"]
        # bf16 -> f32 exactly via bit shift (bf16 is truncated f32)
        yf = (yb.view(np.uint16).astype(np.uint32) << 16).view(np.float32)
        base = 0
        for _, off, m in chunks:
            acc[:, :, off:off + m] += yf[:, base:base + HT * m].reshape(
                128, HT, m)
            base += HT * m
    return np.ascontiguousarray(acc.transpose(1, 0, 2).reshape(H, T).T)


def kernel(permuted_local_hidden_states, weight1, weight2, tokens_per_expert):
    x = np.asarray(permuted_local_hidden_states, np.float32)
    w1 = np.asarray(weight1, np.float32)
    w2 = np.asarray(weight2, np.float32)
    counts = np.asarray(tokens_per_expert).astype(np.int64)

    chunks = _schedule(counts)
    nc = _build(chunks)
    in_maps = _make_inputs(x, w1, w2, chunks)
    res = run_bass_kernel_spmd(nc, in_maps, list(range(NCORES)))
    return _gather(res.results, chunks)
